# revision 19
# baseline (speedup 1.0000x reference)
"""Trainium2 Bass kernel for nn_APLinear (8-bit bit-plane quantized linear).

y = x @ dequant(qweight, scale, zero).T + bias

Sharding: 8 NeuronCores as a 2x4 grid (2 M-groups x 4 N-groups).
Each core computes a [4096, 1024] slice of the [8192, 4096] output:
  - x is split in half along tokens (M); each half is shared by 4 cores.
  - out_features (N) split in quarters; each core dequantizes its own
    weight slice on-chip (u8 -> bf16 affine) and GEMMs it in bf16.

Host-side prep is pure layout: transpose x to [K, M], and re-order the
qweight bit-planes' bytes/bits into a linear [in_features, out_features]
uint8 tensor (a fixed, data-independent permutation of the input bits).
The arithmetic of dequantization (scale/zero application) and the full
GEMM + bias run on-device.
"""

import numpy as np
import ml_dtypes
from contextlib import ExitStack

import concourse.tile as tile
from concourse import bacc, mybir, library_config
from concourse.bass_utils import run_bass_kernel_spmd

BCAST_MODE = "dma"  # "gpsimd": on-chip partition broadcast; "dma": HBM broadcast

# Problem shapes (hardcoded per harness contract)
P = 128
BATCH, SEQ, IN_F, OUT_F = 4, 2048, 4096, 4096
GROUP = 128
W_BITS = 8
MIN_PREC = 4

# Core grid
MG, NG = 2, 4
NCORES = MG * NG
M = (BATCH * SEQ) // MG      # 4096 rows per core
N = OUT_F // NG              # 1024 out features per core
K = IN_F                     # 4096 contraction
KT = K // P                  # 32 k-subtiles (= quant groups)
MC = M // 512                # 8 m-chunks
MS = 4                       # m-subtiles (128) per chunk
NT = N // 512                # 2 n-tiles

_CACHE = {}
LAST_RESULTS = None  # BassKernelResults of the most recent run (for tooling)


# ---------------------------------------------------------------------------
# Host-side layout prep (pure permutation of input bits + tiny param algebra)
# ---------------------------------------------------------------------------

def _make_new_byte_indices(total_bytes: int) -> np.ndarray:
    bytes_per_thread = 4
    threads_per_warp = 32
    bytes_per_warp = threads_per_warp * bytes_per_thread  # 128
    full = total_bytes // bytes_per_warp * bytes_per_warp
    rem = total_bytes - full
    parts = []
    if full > 0:
        bi = np.arange(full)
        warp_offsets = (bi // bytes_per_warp) * bytes_per_warp
        thread_indices = bi % threads_per_warp
        byte_off = ((bi % bytes_per_warp) // threads_per_warp) ^ 3
        parts.append(warp_offsets + thread_indices * bytes_per_thread + byte_off)
    if rem > 0:
        ri = np.arange(rem)
        adj = rem // bytes_per_thread
        if adj > 0:
            warp_offsets = (ri // bytes_per_warp) * bytes_per_warp
            thread_indices = ri % adj
            byte_off = (ri // adj) ^ 3
            parts.append(warp_offsets + thread_indices * 4 + byte_off + full)
        else:
            parts.append(np.arange(full, full + rem))
    return np.concatenate(parts).astype(np.int64)


def _unpack_qweight(qweight: np.ndarray) -> np.ndarray:
    """[W_BITS, OUT, IN//32] int32 -> [OUT, IN] uint8 quantized values.

    Bit-plane k contributes bit (W_BITS-1-k); within each plane the bytes
    go through the fixed warp permutation, bits MSB-first.
    """
    w_bits, n_out, in_chunks = qweight.shape
    total_bytes = in_chunks * 4
    b = qweight[:W_BITS].view(np.uint8).reshape(w_bits, n_out, total_bytes)
    idx = _make_new_byte_indices(total_bytes)
    b = b[:, :, idx]
    bits = np.unpackbits(b, axis=-1, bitorder="big")          # [8, OUT, IN]
    q = np.packbits(np.ascontiguousarray(bits.transpose(1, 2, 0)),
                    axis=-1, bitorder="big")                   # [OUT, IN, 1]
    return q[..., 0]


# ---------------------------------------------------------------------------
# Device graph
# ---------------------------------------------------------------------------

def _emit(nc, tc, ctx, xt, qt, a_ap, b_ap, bias_ap, out_ap, K_, M_, N_):
    f32, bf16, u8 = mybir.dt.float32, mybir.dt.bfloat16, mybir.dt.uint8
    kt, mc_n, nt = K_ // P, M_ // 512, N_ // 512

    wt_pool = ctx.enter_context(tc.tile_pool(name="wt", bufs=1))
    qpool = ctx.enter_context(tc.tile_pool(name="qp", bufs=4))
    qbpool = ctx.enter_context(tc.tile_pool(name="qb", bufs=2))
    bcpool = ctx.enter_context(tc.tile_pool(name="bc", bufs=3))
    tpool = ctx.enter_context(tc.tile_pool(name="tp", bufs=2))
    xbpool = ctx.enter_context(tc.tile_pool(name="xb", bufs=4))
    opool = ctx.enter_context(tc.tile_pool(name="op", bufs=4))
    cpool = ctx.enter_context(tc.tile_pool(name="cp", bufs=1))
    psum = ctx.enter_context(tc.tile_pool(name="ps", bufs=1, space="PSUM"))

    ones = cpool.tile([1, P], bf16)
    nc.any.memset(ones[:], 1.0)
    bias_sb = cpool.tile([1, N_], bf16)
    nc.sync.dma_start(bias_sb[:], bias_ap[:])

    if BCAST_MODE == "gpsimd":
        nc.gpsimd.load_library(library_config.attn)
        rowpool = ctx.enter_context(tc.tile_pool(name="rp", bufs=4))

    # Dequant: wt[g][i, n] = A[g, n] * Q[i, n] + B[g, n]  (bf16)
    # DMAs batched: q two groups per transfer, A/B four groups per transfer.
    wt = [None] * kt
    QB, BB = 2, 2
    qbf_t = [None] * (kt // QB)
    ab_t = [None] * (kt // BB)

    def emit_dequant_group(g):
        if g % BB == 0:
            gb = g // BB
            asb = bcpool.tile([P, BB, N_], bf16, name="asb")
            nc.gpsimd.dma_start(asb[:], a_ap[g:g + BB, :].partition_broadcast(P))
            bsb = bcpool.tile([P, BB, N_], bf16, name="bsb")
            nc.gpsimd.dma_start(bsb[:], b_ap[g:g + BB, :].partition_broadcast(P))
            ab_t[gb] = (asb, bsb)
        if g % QB == 0:
            qsb = qpool.tile([P, QB, N_], u8, name="qsb")
            nc.gpsimd.dma_start(
                qsb[:], qt[g * P:(g + QB) * P, :].rearrange("(ks p) n -> p ks n", p=P))
            qbf = qbpool.tile([P, QB, N_], bf16, name="qbf")
            nc.scalar.copy(qbf[:], qsb[:])
            qbf_t[g // QB] = qbf
        asb, bsb = ab_t[g // BB]
        qbf = qbf_t[g // QB]
        tmp = tpool.tile([P, N_], bf16, name="tmp")
        nc.vector.tensor_tensor(tmp[:], qbf[:, g % QB, :], asb[:, g % BB, :],
                                mybir.AluOpType.mult)
        w_g = wt_pool.tile([P, N_], bf16, name=f"wt{g}")
        nc.vector.tensor_tensor(w_g[:], tmp[:], bsb[:, g % BB, :],
                                mybir.AluOpType.add)
        wt[g] = w_g

    # GEMM: out[m, n] = sum_k xt[k, m] * wt[k, n] (+ bias via K=1 matmul)
    # Dequant of group g is emitted just before chunk 0 consumes it, so the
    # PE ramps as weights stream in instead of waiting for the full dequant.
    for g in range(QB):
        emit_dequant_group(g)
    for mc in range(mc_n):
        ps = [[psum.tile([P, 512], f32, name=f"ps_{mi}_{nj}")
               for nj in range(nt)] for mi in range(MS)]
        for mi in range(MS):
            for nj in range(nt):
                nc.tensor.matmul(ps[mi][nj][:], ones[:],
                                 bias_sb[0:1, nj * 512:(nj + 1) * 512],
                                 start=True, stop=False)
        XB = 4
        for gb in range(kt // XB):
            if mc == 0:
                for ks in range(XB):
                    g2 = gb * XB + ks + QB
                    if g2 < kt:
                        emit_dequant_group(g2)
            xb = xbpool.tile([P, XB, 512], bf16, name="xb")
            nc.sync.dma_start(
                xb[:], xt[gb * XB * P:(gb + 1) * XB * P,
                          mc * 512:(mc + 1) * 512]
                .rearrange("(ks p) m -> p ks m", p=P))
            for ks in range(XB):
                g = gb * XB + ks
                last = g == kt - 1
                for mi in range(MS):
                    for nj in range(nt):
                        nc.tensor.matmul(ps[mi][nj][:],
                                         xb[:, ks, mi * P:(mi + 1) * P],
                                         wt[g][:, nj * 512:(nj + 1) * 512],
                                         start=False, stop=last)
        if mc == mc_n - 1:
            for mi in range(MS):
                for nj in range(nt):
                    o2 = opool.tile([P, 512], f32, name="o2")
                    if (mi + nj) % 2 == 0:
                        nc.scalar.copy(o2[:], ps[mi][nj][:])
                    else:
                        nc.vector.tensor_copy(o2[:], ps[mi][nj][:])
                    nc.sync.dma_start(
                        out_ap[mc * 512 + mi * P: mc * 512 + (mi + 1) * P,
                               nj * 512:(nj + 1) * 512], o2[:])
        else:
            for mi in range(MS):
                o = opool.tile([P, N_], f32, name="o")
                for nj in range(nt):
                    if nj % 2 == 0:
                        nc.scalar.copy(o[:, nj * 512:(nj + 1) * 512], ps[mi][nj][:])
                    else:
                        nc.vector.tensor_copy(o[:, nj * 512:(nj + 1) * 512],
                                              ps[mi][nj][:])
                nc.sync.dma_start(
                    out_ap[mc * 512 + mi * P: mc * 512 + (mi + 1) * P, :], o[:])


def _build_graph():
    nc = bacc.Bacc("TRN2", target_bir_lowering=False, debug=False,
                   num_devices=NCORES)
    f32, bf16, u8 = mybir.dt.float32, mybir.dt.bfloat16, mybir.dt.uint8
    xt_t = nc.dram_tensor("xt", [K, M], bf16, kind="ExternalInput")
    qt_t = nc.dram_tensor("qt", [K, N], u8, kind="ExternalInput")
    a_t = nc.dram_tensor("scl", [KT, N], bf16, kind="ExternalInput")
    b_t = nc.dram_tensor("sclz", [KT, N], bf16, kind="ExternalInput")
    bias_t = nc.dram_tensor("bias", [1, N], bf16, kind="ExternalInput")
    out_t = nc.dram_tensor("out", [M, N], f32, kind="ExternalOutput")

    with tile.TileContext(nc) as tc:
        with ExitStack() as ctx:
            _emit(nc, tc, ctx, xt_t.ap(), qt_t.ap(), a_t.ap(), b_t.ap(),
                  bias_t.ap(), out_t.ap(), K, M, N)
    nc.compile()
    return nc


def _get_graph():
    if "nc" not in _CACHE:
        _CACHE["nc"] = _build_graph()
    return _CACHE["nc"]


# ---------------------------------------------------------------------------
# Entry point
# ---------------------------------------------------------------------------

def kernel(x, qweight, scale, zero, bias, _trace=False, _trace_cores=None):
    global LAST_RESULTS
    x = np.ascontiguousarray(np.asarray(x, dtype=np.float32))
    qweight = np.ascontiguousarray(np.asarray(qweight, dtype=np.int32))
    scale = np.asarray(scale, dtype=np.float32)
    zero = np.asarray(zero, dtype=np.float32)
    bias = np.asarray(bias, dtype=np.float32)

    # Host layout prep
    xf = x.reshape(BATCH * SEQ, IN_F)
    q = _unpack_qweight(qweight)                  # [OUT, IN] u8
    qt_full = np.ascontiguousarray(q.T)           # [IN, OUT] u8
    a_full = np.ascontiguousarray(scale.T).astype(ml_dtypes.bfloat16)  # [32, OUT]
    b_full = np.ascontiguousarray(
        (-(2.0 ** (W_BITS - MIN_PREC)) * scale * zero).T
    ).astype(ml_dtypes.bfloat16)                  # [32, OUT]
    bias_bf = bias.astype(ml_dtypes.bfloat16)

    xt_halves = [np.ascontiguousarray(xf[mg * M:(mg + 1) * M].T)
                 .astype(ml_dtypes.bfloat16)
                 for mg in range(MG)]             # each [K, M] bf16

    in_maps = []
    for c in range(NCORES):
        mg, ng = divmod(c, NG)
        nsl = slice(ng * N, (ng + 1) * N)
        in_maps.append({
            "xt": xt_halves[mg],
            "qt": np.ascontiguousarray(qt_full[:, nsl]),
            "scl": np.ascontiguousarray(a_full[:, nsl]),
            "sclz": np.ascontiguousarray(b_full[:, nsl]),
            "bias": np.ascontiguousarray(bias_bf[nsl].reshape(1, N)),
        })

    nc = _get_graph()
    res = run_bass_kernel_spmd(nc, in_maps, core_ids=list(range(NCORES)),
                               trace=_trace, trace_cores=_trace_cores)
    LAST_RESULTS = res

    out_full = np.empty((BATCH * SEQ, OUT_F), dtype=np.float32)
    for c in range(NCORES):
        mg, ng = divmod(c, NG)
        out_full[mg * M:(mg + 1) * M, ng * N:(ng + 1) * N] = res.results[c]["out"]
    return out_full.reshape(BATCH, SEQ, OUT_F)


# revision 21
# speedup vs baseline: 1.0161x; 1.0161x over previous
"""Trainium2 Bass kernel for nn_APLinear (8-bit bit-plane quantized linear).

y = x @ dequant(qweight, scale, zero).T + bias

Sharding: 8 NeuronCores as a 2x4 grid (2 M-groups x 4 N-groups).
Each core computes a [4096, 1024] slice of the [8192, 4096] output:
  - x is split in half along tokens (M); each half is shared by 4 cores.
  - out_features (N) split in quarters; each core dequantizes its own
    weight slice on-chip (u8 -> bf16 affine) and GEMMs it in bf16.

Host-side prep is pure layout: transpose x to [K, M], and re-order the
qweight bit-planes' bytes/bits into a linear [in_features, out_features]
uint8 tensor (a fixed, data-independent permutation of the input bits).
The arithmetic of dequantization (scale/zero application) and the full
GEMM + bias run on-device.
"""

import numpy as np
import ml_dtypes
from contextlib import ExitStack

import concourse.tile as tile
from concourse import bacc, mybir, library_config
from concourse.bass_utils import run_bass_kernel_spmd

BCAST_MODE = "dma"  # "gpsimd": on-chip partition broadcast; "dma": HBM broadcast

# Problem shapes (hardcoded per harness contract)
P = 128
BATCH, SEQ, IN_F, OUT_F = 4, 2048, 4096, 4096
GROUP = 128
W_BITS = 8
MIN_PREC = 4

# Core grid
MG, NG = 2, 4
NCORES = MG * NG
M = (BATCH * SEQ) // MG      # 4096 rows per core
N = OUT_F // NG              # 1024 out features per core
K = IN_F                     # 4096 contraction
KT = K // P                  # 32 k-subtiles (= quant groups)
MC = M // 512                # 8 m-chunks
MS = 4                       # m-subtiles (128) per chunk
NT = N // 512                # 2 n-tiles

_CACHE = {}
LAST_RESULTS = None  # BassKernelResults of the most recent run (for tooling)


# ---------------------------------------------------------------------------
# Host-side layout prep (pure permutation of input bits + tiny param algebra)
# ---------------------------------------------------------------------------

def _make_new_byte_indices(total_bytes: int) -> np.ndarray:
    bytes_per_thread = 4
    threads_per_warp = 32
    bytes_per_warp = threads_per_warp * bytes_per_thread  # 128
    full = total_bytes // bytes_per_warp * bytes_per_warp
    rem = total_bytes - full
    parts = []
    if full > 0:
        bi = np.arange(full)
        warp_offsets = (bi // bytes_per_warp) * bytes_per_warp
        thread_indices = bi % threads_per_warp
        byte_off = ((bi % bytes_per_warp) // threads_per_warp) ^ 3
        parts.append(warp_offsets + thread_indices * bytes_per_thread + byte_off)
    if rem > 0:
        ri = np.arange(rem)
        adj = rem // bytes_per_thread
        if adj > 0:
            warp_offsets = (ri // bytes_per_warp) * bytes_per_warp
            thread_indices = ri % adj
            byte_off = (ri // adj) ^ 3
            parts.append(warp_offsets + thread_indices * 4 + byte_off + full)
        else:
            parts.append(np.arange(full, full + rem))
    return np.concatenate(parts).astype(np.int64)


def _unpack_qweight(qweight: np.ndarray) -> np.ndarray:
    """[W_BITS, OUT, IN//32] int32 -> [OUT, IN] uint8 quantized values.

    Bit-plane k contributes bit (W_BITS-1-k); within each plane the bytes
    go through the fixed warp permutation, bits MSB-first.
    """
    w_bits, n_out, in_chunks = qweight.shape
    total_bytes = in_chunks * 4
    b = qweight[:W_BITS].view(np.uint8).reshape(w_bits, n_out, total_bytes)
    idx = _make_new_byte_indices(total_bytes)
    b = b[:, :, idx]
    bits = np.unpackbits(b, axis=-1, bitorder="big")          # [8, OUT, IN]
    q = np.packbits(np.ascontiguousarray(bits.transpose(1, 2, 0)),
                    axis=-1, bitorder="big")                   # [OUT, IN, 1]
    return q[..., 0]


# ---------------------------------------------------------------------------
# Device graph
# ---------------------------------------------------------------------------

def _emit(nc, tc, ctx, xt, qt, a_ap, b_ap, bias_ap, out_ap, K_, M_, N_):
    f32, bf16, u8 = mybir.dt.float32, mybir.dt.bfloat16, mybir.dt.uint8
    kt, mc_n, nt = K_ // P, M_ // 512, N_ // 512

    wt_pool = ctx.enter_context(tc.tile_pool(name="wt", bufs=1))
    qpool = ctx.enter_context(tc.tile_pool(name="qp", bufs=4))
    qbpool = ctx.enter_context(tc.tile_pool(name="qb", bufs=2))
    bcpool = ctx.enter_context(tc.tile_pool(name="bc", bufs=3))
    tpool = ctx.enter_context(tc.tile_pool(name="tp", bufs=2))
    xbpool = ctx.enter_context(tc.tile_pool(name="xb", bufs=4))
    opool = ctx.enter_context(tc.tile_pool(name="op", bufs=4))
    cpool = ctx.enter_context(tc.tile_pool(name="cp", bufs=1))
    psum = ctx.enter_context(tc.tile_pool(name="ps", bufs=1, space="PSUM"))

    bias_bc = cpool.tile([P, N_], f32)
    nc.sync.dma_start(bias_bc[:], bias_ap[0:1, :].partition_broadcast(P))

    if BCAST_MODE == "gpsimd":
        nc.gpsimd.load_library(library_config.attn)
        rowpool = ctx.enter_context(tc.tile_pool(name="rp", bufs=4))

    # Dequant: wt[g][i, n] = A[g, n] * Q[i, n] + B[g, n]  (bf16)
    # DMAs batched: q two groups per transfer, A/B four groups per transfer.
    wt = [None] * kt
    QB, BB = 2, 2
    qbf_t = [None] * (kt // QB)
    ab_t = [None] * (kt // BB)

    def emit_dequant_group(g):
        if g % BB == 0:
            gb = g // BB
            asb = bcpool.tile([P, BB, N_], bf16, name="asb")
            nc.gpsimd.dma_start(asb[:], a_ap[g:g + BB, :].partition_broadcast(P))
            bsb = bcpool.tile([P, BB, N_], bf16, name="bsb")
            nc.gpsimd.dma_start(bsb[:], b_ap[g:g + BB, :].partition_broadcast(P))
            ab_t[gb] = (asb, bsb)
        if g % QB == 0:
            qsb = qpool.tile([P, QB, N_], u8, name="qsb")
            nc.gpsimd.dma_start(
                qsb[:], qt[g * P:(g + QB) * P, :].rearrange("(ks p) n -> p ks n", p=P))
            qbf = qbpool.tile([P, QB, N_], bf16, name="qbf")
            nc.scalar.copy(qbf[:], qsb[:])
            qbf_t[g // QB] = qbf
        asb, bsb = ab_t[g // BB]
        qbf = qbf_t[g // QB]
        tmp = tpool.tile([P, N_], bf16, name="tmp")
        nc.vector.tensor_tensor(tmp[:], qbf[:, g % QB, :], asb[:, g % BB, :],
                                mybir.AluOpType.mult)
        w_g = wt_pool.tile([P, N_], bf16, name=f"wt{g}")
        nc.vector.tensor_tensor(w_g[:], tmp[:], bsb[:, g % BB, :],
                                mybir.AluOpType.add)
        wt[g] = w_g

    # GEMM: out[m, n] = sum_k xt[k, m] * wt[k, n] (+ bias via K=1 matmul)
    # Dequant of group g is emitted just before chunk 0 consumes it, so the
    # PE ramps as weights stream in instead of waiting for the full dequant.
    for g in range(QB):
        emit_dequant_group(g)
    for mc in range(mc_n):
        ps = [[psum.tile([P, 512], f32, name=f"ps_{mi}_{nj}")
               for nj in range(nt)] for mi in range(MS)]
        XB = 4
        for gb in range(kt // XB):
            if mc == 0:
                for ks in range(XB):
                    g2 = gb * XB + ks + QB
                    if g2 < kt:
                        emit_dequant_group(g2)
            xb = xbpool.tile([P, XB, 512], bf16, name="xb")
            nc.sync.dma_start(
                xb[:], xt[gb * XB * P:(gb + 1) * XB * P,
                          mc * 512:(mc + 1) * 512]
                .rearrange("(ks p) m -> p ks m", p=P))
            for ks in range(XB):
                g = gb * XB + ks
                last = g == kt - 1
                for mi in range(MS):
                    for nj in range(nt):
                        nc.tensor.matmul(ps[mi][nj][:],
                                         xb[:, ks, mi * P:(mi + 1) * P],
                                         wt[g][:, nj * 512:(nj + 1) * 512],
                                         start=(g == 0), stop=last)
        if mc == mc_n - 1:
            for mi in range(MS):
                for nj in range(nt):
                    o2 = opool.tile([P, 512], f32, name="o2")
                    nc.vector.tensor_tensor(o2[:], ps[mi][nj][:],
                                            bias_bc[:, nj * 512:(nj + 1) * 512],
                                            mybir.AluOpType.add)
                    nc.sync.dma_start(
                        out_ap[mc * 512 + mi * P: mc * 512 + (mi + 1) * P,
                               nj * 512:(nj + 1) * 512], o2[:])
        else:
            for mi in range(MS):
                o = opool.tile([P, N_], f32, name="o")
                for nj in range(nt):
                    nc.vector.tensor_tensor(o[:, nj * 512:(nj + 1) * 512],
                                            ps[mi][nj][:],
                                            bias_bc[:, nj * 512:(nj + 1) * 512],
                                            mybir.AluOpType.add)
                nc.sync.dma_start(
                    out_ap[mc * 512 + mi * P: mc * 512 + (mi + 1) * P, :], o[:])


def _build_graph():
    nc = bacc.Bacc("TRN2", target_bir_lowering=False, debug=False,
                   num_devices=NCORES)
    f32, bf16, u8 = mybir.dt.float32, mybir.dt.bfloat16, mybir.dt.uint8
    xt_t = nc.dram_tensor("xt", [K, M], bf16, kind="ExternalInput")
    qt_t = nc.dram_tensor("qt", [K, N], u8, kind="ExternalInput")
    a_t = nc.dram_tensor("scl", [KT, N], bf16, kind="ExternalInput")
    b_t = nc.dram_tensor("sclz", [KT, N], bf16, kind="ExternalInput")
    bias_t = nc.dram_tensor("bias", [1, N], f32, kind="ExternalInput")
    out_t = nc.dram_tensor("out", [M, N], f32, kind="ExternalOutput")

    with tile.TileContext(nc) as tc:
        with ExitStack() as ctx:
            _emit(nc, tc, ctx, xt_t.ap(), qt_t.ap(), a_t.ap(), b_t.ap(),
                  bias_t.ap(), out_t.ap(), K, M, N)
    nc.compile()
    return nc


def _get_graph():
    if "nc" not in _CACHE:
        _CACHE["nc"] = _build_graph()
    return _CACHE["nc"]


# ---------------------------------------------------------------------------
# Entry point
# ---------------------------------------------------------------------------

def kernel(x, qweight, scale, zero, bias, _trace=False, _trace_cores=None):
    global LAST_RESULTS
    x = np.ascontiguousarray(np.asarray(x, dtype=np.float32))
    qweight = np.ascontiguousarray(np.asarray(qweight, dtype=np.int32))
    scale = np.asarray(scale, dtype=np.float32)
    zero = np.asarray(zero, dtype=np.float32)
    bias = np.asarray(bias, dtype=np.float32)

    # Host layout prep
    xf = x.reshape(BATCH * SEQ, IN_F)
    q = _unpack_qweight(qweight)                  # [OUT, IN] u8
    qt_full = np.ascontiguousarray(q.T)           # [IN, OUT] u8
    a_full = np.ascontiguousarray(scale.T).astype(ml_dtypes.bfloat16)  # [32, OUT]
    b_full = np.ascontiguousarray(
        (-(2.0 ** (W_BITS - MIN_PREC)) * scale * zero).T
    ).astype(ml_dtypes.bfloat16)                  # [32, OUT]


    xt_halves = [np.ascontiguousarray(xf[mg * M:(mg + 1) * M].T)
                 .astype(ml_dtypes.bfloat16)
                 for mg in range(MG)]             # each [K, M] bf16

    in_maps = []
    for c in range(NCORES):
        mg, ng = divmod(c, NG)
        nsl = slice(ng * N, (ng + 1) * N)
        in_maps.append({
            "xt": xt_halves[mg],
            "qt": np.ascontiguousarray(qt_full[:, nsl]),
            "scl": np.ascontiguousarray(a_full[:, nsl]),
            "sclz": np.ascontiguousarray(b_full[:, nsl]),
            "bias": np.ascontiguousarray(bias[nsl].reshape(1, N)),
        })

    nc = _get_graph()
    res = run_bass_kernel_spmd(nc, in_maps, core_ids=list(range(NCORES)),
                               trace=_trace, trace_cores=_trace_cores)
    LAST_RESULTS = res

    out_full = np.empty((BATCH * SEQ, OUT_F), dtype=np.float32)
    for c in range(NCORES):
        mg, ng = divmod(c, NG)
        out_full[mg * M:(mg + 1) * M, ng * N:(ng + 1) * N] = res.results[c]["out"]
    return out_full.reshape(BATCH, SEQ, OUT_F)


# revision 22
# speedup vs baseline: 1.0261x; 1.0098x over previous
"""Trainium2 Bass kernel for nn_APLinear (8-bit bit-plane quantized linear).

y = x @ dequant(qweight, scale, zero).T + bias

Sharding: 8 NeuronCores as a 2x4 grid (2 M-groups x 4 N-groups).
Each core computes a [4096, 1024] slice of the [8192, 4096] output:
  - x is split in half along tokens (M); each half is shared by 4 cores.
  - out_features (N) split in quarters; each core dequantizes its own
    weight slice on-chip (u8 -> bf16 affine) and GEMMs it in bf16.

Host-side prep is pure layout: transpose x to [K, M], and re-order the
qweight bit-planes' bytes/bits into a linear [in_features, out_features]
uint8 tensor (a fixed, data-independent permutation of the input bits).
The arithmetic of dequantization (scale/zero application) and the full
GEMM + bias run on-device.
"""

import numpy as np
import ml_dtypes
from contextlib import ExitStack

import concourse.tile as tile
from concourse import bacc, mybir, library_config
from concourse.bass_utils import run_bass_kernel_spmd

BCAST_MODE = "dma"  # "gpsimd": on-chip partition broadcast; "dma": HBM broadcast

# Problem shapes (hardcoded per harness contract)
P = 128
BATCH, SEQ, IN_F, OUT_F = 4, 2048, 4096, 4096
GROUP = 128
W_BITS = 8
MIN_PREC = 4

# Core grid
MG, NG = 2, 4
NCORES = MG * NG
M = (BATCH * SEQ) // MG      # 4096 rows per core
N = OUT_F // NG              # 1024 out features per core
K = IN_F                     # 4096 contraction
KT = K // P                  # 32 k-subtiles (= quant groups)
MC = M // 512                # 8 m-chunks
MS = 4                       # m-subtiles (128) per chunk
NT = N // 512                # 2 n-tiles

_CACHE = {}
LAST_RESULTS = None  # BassKernelResults of the most recent run (for tooling)


# ---------------------------------------------------------------------------
# Host-side layout prep (pure permutation of input bits + tiny param algebra)
# ---------------------------------------------------------------------------

def _make_new_byte_indices(total_bytes: int) -> np.ndarray:
    bytes_per_thread = 4
    threads_per_warp = 32
    bytes_per_warp = threads_per_warp * bytes_per_thread  # 128
    full = total_bytes // bytes_per_warp * bytes_per_warp
    rem = total_bytes - full
    parts = []
    if full > 0:
        bi = np.arange(full)
        warp_offsets = (bi // bytes_per_warp) * bytes_per_warp
        thread_indices = bi % threads_per_warp
        byte_off = ((bi % bytes_per_warp) // threads_per_warp) ^ 3
        parts.append(warp_offsets + thread_indices * bytes_per_thread + byte_off)
    if rem > 0:
        ri = np.arange(rem)
        adj = rem // bytes_per_thread
        if adj > 0:
            warp_offsets = (ri // bytes_per_warp) * bytes_per_warp
            thread_indices = ri % adj
            byte_off = (ri // adj) ^ 3
            parts.append(warp_offsets + thread_indices * 4 + byte_off + full)
        else:
            parts.append(np.arange(full, full + rem))
    return np.concatenate(parts).astype(np.int64)


def _unpack_qweight(qweight: np.ndarray) -> np.ndarray:
    """[W_BITS, OUT, IN//32] int32 -> [OUT, IN] uint8 quantized values.

    Bit-plane k contributes bit (W_BITS-1-k); within each plane the bytes
    go through the fixed warp permutation, bits MSB-first.
    """
    w_bits, n_out, in_chunks = qweight.shape
    total_bytes = in_chunks * 4
    b = qweight[:W_BITS].view(np.uint8).reshape(w_bits, n_out, total_bytes)
    idx = _make_new_byte_indices(total_bytes)
    b = b[:, :, idx]
    bits = np.unpackbits(b, axis=-1, bitorder="big")          # [8, OUT, IN]
    q = np.packbits(np.ascontiguousarray(bits.transpose(1, 2, 0)),
                    axis=-1, bitorder="big")                   # [OUT, IN, 1]
    return q[..., 0]


# ---------------------------------------------------------------------------
# Device graph
# ---------------------------------------------------------------------------

def _emit(nc, tc, ctx, xt, qt, a_ap, b_ap, bias_ap, out_ap, K_, M_, N_):
    f32, bf16, u8 = mybir.dt.float32, mybir.dt.bfloat16, mybir.dt.uint8
    kt, mc_n, nt = K_ // P, M_ // 512, N_ // 512

    wt_pool = ctx.enter_context(tc.tile_pool(name="wt", bufs=1))
    qpool = ctx.enter_context(tc.tile_pool(name="qp", bufs=4))
    qbpool = ctx.enter_context(tc.tile_pool(name="qb", bufs=3))
    bcpool = ctx.enter_context(tc.tile_pool(name="bc", bufs=4))
    tpool = ctx.enter_context(tc.tile_pool(name="tp", bufs=2))
    xbpool = ctx.enter_context(tc.tile_pool(name="xb", bufs=4))
    opool = ctx.enter_context(tc.tile_pool(name="op", bufs=4))
    cpool = ctx.enter_context(tc.tile_pool(name="cp", bufs=1))
    psum = ctx.enter_context(tc.tile_pool(name="ps", bufs=1, space="PSUM"))

    bias_bc = cpool.tile([P, N_], f32)

    if BCAST_MODE == "gpsimd":
        nc.gpsimd.load_library(library_config.attn)
        rowpool = ctx.enter_context(tc.tile_pool(name="rp", bufs=4))

    # Dequant: wt[g][i, n] = A[g, n] * Q[i, n] + B[g, n]  (bf16)
    # DMAs batched: q two groups per transfer, A/B four groups per transfer.
    wt = [None] * kt
    QB, BB = 2, 2
    qbf_t = [None] * (kt // QB)
    ab_t = [None] * (kt // BB)

    def emit_dequant_group(g):
        if g % BB == 0:
            gb = g // BB
            asb = bcpool.tile([P, BB, N_], bf16, name="asb")
            nc.gpsimd.dma_start(asb[:], a_ap[g:g + BB, :].partition_broadcast(P))
            bsb = bcpool.tile([P, BB, N_], bf16, name="bsb")
            nc.gpsimd.dma_start(bsb[:], b_ap[g:g + BB, :].partition_broadcast(P))
            ab_t[gb] = (asb, bsb)
        if g % QB == 0:
            qsb = qpool.tile([P, QB, N_], u8, name="qsb")
            nc.gpsimd.dma_start(
                qsb[:], qt[g * P:(g + QB) * P, :].rearrange("(ks p) n -> p ks n", p=P))
            qbf = qbpool.tile([P, QB, N_], bf16, name="qbf")
            for ks2 in range(QB):
                nc.scalar.copy(qbf[:, ks2, :], qsb[:, ks2, :])
            qbf_t[g // QB] = qbf
        asb, bsb = ab_t[g // BB]
        qbf = qbf_t[g // QB]
        tmp = tpool.tile([P, N_], bf16, name="tmp")
        nc.vector.tensor_tensor(tmp[:], qbf[:, g % QB, :], asb[:, g % BB, :],
                                mybir.AluOpType.mult)
        w_g = wt_pool.tile([P, N_], bf16, name=f"wt{g}")
        nc.vector.tensor_tensor(w_g[:], tmp[:], bsb[:, g % BB, :],
                                mybir.AluOpType.add)
        wt[g] = w_g

    # GEMM: out[m, n] = sum_k xt[k, m] * wt[k, n] (+ bias via K=1 matmul)
    # Dequant of group g is emitted just before chunk 0 consumes it, so the
    # PE ramps as weights stream in instead of waiting for the full dequant.
    for g in range(QB):
        emit_dequant_group(g)
    nc.sync.dma_start(bias_bc[:], bias_ap[0:1, :].partition_broadcast(P))
    for mc in range(mc_n):
        ps = [[psum.tile([P, 512], f32, name=f"ps_{mi}_{nj}")
               for nj in range(nt)] for mi in range(MS)]
        XB = 4
        for gb in range(kt // XB):
            if mc == 0:
                for ks in range(XB):
                    g2 = gb * XB + ks + QB
                    if g2 < kt:
                        emit_dequant_group(g2)
            xb = xbpool.tile([P, XB, 512], bf16, name="xb")
            nc.sync.dma_start(
                xb[:], xt[gb * XB * P:(gb + 1) * XB * P,
                          mc * 512:(mc + 1) * 512]
                .rearrange("(ks p) m -> p ks m", p=P))
            for ks in range(XB):
                g = gb * XB + ks
                last = g == kt - 1
                for mi in range(MS):
                    for nj in range(nt):
                        nc.tensor.matmul(ps[mi][nj][:],
                                         xb[:, ks, mi * P:(mi + 1) * P],
                                         wt[g][:, nj * 512:(nj + 1) * 512],
                                         start=(g == 0), stop=last)
        if mc == mc_n - 1:
            for mi in range(MS):
                for nj in range(nt):
                    o2 = opool.tile([P, 512], f32, name="o2")
                    nc.vector.tensor_tensor(o2[:], ps[mi][nj][:],
                                            bias_bc[:, nj * 512:(nj + 1) * 512],
                                            mybir.AluOpType.add)
                    nc.sync.dma_start(
                        out_ap[mc * 512 + mi * P: mc * 512 + (mi + 1) * P,
                               nj * 512:(nj + 1) * 512], o2[:])
        else:
            for mi in range(MS):
                o = opool.tile([P, N_], f32, name="o")
                for nj in range(nt):
                    nc.vector.tensor_tensor(o[:, nj * 512:(nj + 1) * 512],
                                            ps[mi][nj][:],
                                            bias_bc[:, nj * 512:(nj + 1) * 512],
                                            mybir.AluOpType.add)
                nc.sync.dma_start(
                    out_ap[mc * 512 + mi * P: mc * 512 + (mi + 1) * P, :], o[:])


def _build_graph():
    nc = bacc.Bacc("TRN2", target_bir_lowering=False, debug=False,
                   num_devices=NCORES)
    f32, bf16, u8 = mybir.dt.float32, mybir.dt.bfloat16, mybir.dt.uint8
    xt_t = nc.dram_tensor("xt", [K, M], bf16, kind="ExternalInput")
    qt_t = nc.dram_tensor("qt", [K, N], u8, kind="ExternalInput")
    a_t = nc.dram_tensor("scl", [KT, N], bf16, kind="ExternalInput")
    b_t = nc.dram_tensor("sclz", [KT, N], bf16, kind="ExternalInput")
    bias_t = nc.dram_tensor("bias", [1, N], f32, kind="ExternalInput")
    out_t = nc.dram_tensor("out", [M, N], f32, kind="ExternalOutput")

    with tile.TileContext(nc) as tc:
        with ExitStack() as ctx:
            _emit(nc, tc, ctx, xt_t.ap(), qt_t.ap(), a_t.ap(), b_t.ap(),
                  bias_t.ap(), out_t.ap(), K, M, N)
    nc.compile()
    return nc


def _get_graph():
    if "nc" not in _CACHE:
        _CACHE["nc"] = _build_graph()
    return _CACHE["nc"]


# ---------------------------------------------------------------------------
# Entry point
# ---------------------------------------------------------------------------

def kernel(x, qweight, scale, zero, bias, _trace=False, _trace_cores=None):
    global LAST_RESULTS
    x = np.ascontiguousarray(np.asarray(x, dtype=np.float32))
    qweight = np.ascontiguousarray(np.asarray(qweight, dtype=np.int32))
    scale = np.asarray(scale, dtype=np.float32)
    zero = np.asarray(zero, dtype=np.float32)
    bias = np.asarray(bias, dtype=np.float32)

    # Host layout prep
    xf = x.reshape(BATCH * SEQ, IN_F)
    q = _unpack_qweight(qweight)                  # [OUT, IN] u8
    qt_full = np.ascontiguousarray(q.T)           # [IN, OUT] u8
    a_full = np.ascontiguousarray(scale.T).astype(ml_dtypes.bfloat16)  # [32, OUT]
    b_full = np.ascontiguousarray(
        (-(2.0 ** (W_BITS - MIN_PREC)) * scale * zero).T
    ).astype(ml_dtypes.bfloat16)                  # [32, OUT]


    xt_halves = [np.ascontiguousarray(xf[mg * M:(mg + 1) * M].T)
                 .astype(ml_dtypes.bfloat16)
                 for mg in range(MG)]             # each [K, M] bf16

    in_maps = []
    for c in range(NCORES):
        mg, ng = divmod(c, NG)
        nsl = slice(ng * N, (ng + 1) * N)
        in_maps.append({
            "xt": xt_halves[mg],
            "qt": np.ascontiguousarray(qt_full[:, nsl]),
            "scl": np.ascontiguousarray(a_full[:, nsl]),
            "sclz": np.ascontiguousarray(b_full[:, nsl]),
            "bias": np.ascontiguousarray(bias[nsl].reshape(1, N)),
        })

    nc = _get_graph()
    res = run_bass_kernel_spmd(nc, in_maps, core_ids=list(range(NCORES)),
                               trace=_trace, trace_cores=_trace_cores)
    LAST_RESULTS = res

    out_full = np.empty((BATCH * SEQ, OUT_F), dtype=np.float32)
    for c in range(NCORES):
        mg, ng = divmod(c, NG)
        out_full[mg * M:(mg + 1) * M, ng * N:(ng + 1) * N] = res.results[c]["out"]
    return out_full.reshape(BATCH, SEQ, OUT_F)


# revision 23
# speedup vs baseline: 1.0381x; 1.0117x over previous
"""Trainium2 Bass kernel for nn_APLinear (8-bit bit-plane quantized linear).

y = x @ dequant(qweight, scale, zero).T + bias

Sharding: 8 NeuronCores as a 2x4 grid (2 M-groups x 4 N-groups).
Each core computes a [4096, 1024] slice of the [8192, 4096] output:
  - x is split in half along tokens (M); each half is shared by 4 cores.
  - out_features (N) split in quarters; each core dequantizes its own
    weight slice on-chip (u8 -> bf16 affine) and GEMMs it in bf16.

Host-side prep is pure layout: transpose x to [K, M], and re-order the
qweight bit-planes' bytes/bits into a linear [in_features, out_features]
uint8 tensor (a fixed, data-independent permutation of the input bits).
The arithmetic of dequantization (scale/zero application) and the full
GEMM + bias run on-device.
"""

import numpy as np
import ml_dtypes
from contextlib import ExitStack

import concourse.tile as tile
from concourse import bacc, mybir, library_config
from concourse.bass_utils import run_bass_kernel_spmd

BCAST_MODE = "dma"  # "gpsimd": on-chip partition broadcast; "dma": HBM broadcast

# Problem shapes (hardcoded per harness contract)
P = 128
BATCH, SEQ, IN_F, OUT_F = 4, 2048, 4096, 4096
GROUP = 128
W_BITS = 8
MIN_PREC = 4

# Core grid
MG, NG = 2, 4
NCORES = MG * NG
M = (BATCH * SEQ) // MG      # 4096 rows per core
N = OUT_F // NG              # 1024 out features per core
K = IN_F                     # 4096 contraction
KT = K // P                  # 32 k-subtiles (= quant groups)
MC = M // 512                # 8 m-chunks
MS = 4                       # m-subtiles (128) per chunk
NT = N // 512                # 2 n-tiles

_CACHE = {}
LAST_RESULTS = None  # BassKernelResults of the most recent run (for tooling)


# ---------------------------------------------------------------------------
# Host-side layout prep (pure permutation of input bits + tiny param algebra)
# ---------------------------------------------------------------------------

def _make_new_byte_indices(total_bytes: int) -> np.ndarray:
    bytes_per_thread = 4
    threads_per_warp = 32
    bytes_per_warp = threads_per_warp * bytes_per_thread  # 128
    full = total_bytes // bytes_per_warp * bytes_per_warp
    rem = total_bytes - full
    parts = []
    if full > 0:
        bi = np.arange(full)
        warp_offsets = (bi // bytes_per_warp) * bytes_per_warp
        thread_indices = bi % threads_per_warp
        byte_off = ((bi % bytes_per_warp) // threads_per_warp) ^ 3
        parts.append(warp_offsets + thread_indices * bytes_per_thread + byte_off)
    if rem > 0:
        ri = np.arange(rem)
        adj = rem // bytes_per_thread
        if adj > 0:
            warp_offsets = (ri // bytes_per_warp) * bytes_per_warp
            thread_indices = ri % adj
            byte_off = (ri // adj) ^ 3
            parts.append(warp_offsets + thread_indices * 4 + byte_off + full)
        else:
            parts.append(np.arange(full, full + rem))
    return np.concatenate(parts).astype(np.int64)


def _unpack_qweight(qweight: np.ndarray) -> np.ndarray:
    """[W_BITS, OUT, IN//32] int32 -> [OUT, IN] uint8 quantized values.

    Bit-plane k contributes bit (W_BITS-1-k); within each plane the bytes
    go through the fixed warp permutation, bits MSB-first.
    """
    w_bits, n_out, in_chunks = qweight.shape
    total_bytes = in_chunks * 4
    b = qweight[:W_BITS].view(np.uint8).reshape(w_bits, n_out, total_bytes)
    idx = _make_new_byte_indices(total_bytes)
    b = b[:, :, idx]
    bits = np.unpackbits(b, axis=-1, bitorder="big")          # [8, OUT, IN]
    q = np.packbits(np.ascontiguousarray(bits.transpose(1, 2, 0)),
                    axis=-1, bitorder="big")                   # [OUT, IN, 1]
    return q[..., 0]


# ---------------------------------------------------------------------------
# Device graph
# ---------------------------------------------------------------------------

def _emit(nc, tc, ctx, xt, qt, a_ap, b_ap, bias_ap, out_ap, K_, M_, N_):
    f32, bf16, u8 = mybir.dt.float32, mybir.dt.bfloat16, mybir.dt.uint8
    kt, mc_n, nt = K_ // P, M_ // 512, N_ // 512

    wt_pool = ctx.enter_context(tc.tile_pool(name="wt", bufs=1))
    qpool = ctx.enter_context(tc.tile_pool(name="qp", bufs=4))
    qbpool = ctx.enter_context(tc.tile_pool(name="qb", bufs=3))
    bcpool = ctx.enter_context(tc.tile_pool(name="bc", bufs=3))
    tpool = ctx.enter_context(tc.tile_pool(name="tp", bufs=2))
    xbpool = ctx.enter_context(tc.tile_pool(name="xb", bufs=4))
    opool = ctx.enter_context(tc.tile_pool(name="op", bufs=4))
    cpool = ctx.enter_context(tc.tile_pool(name="cp", bufs=1))
    psum = ctx.enter_context(tc.tile_pool(name="ps", bufs=1, space="PSUM"))

    bias_bc = cpool.tile([P, N_], f32)

    if BCAST_MODE == "gpsimd":
        nc.gpsimd.load_library(library_config.attn)
        rowpool = ctx.enter_context(tc.tile_pool(name="rp", bufs=4))

    # Dequant: wt[g][i, n] = A[g, n] * Q[i, n] + B[g, n]  (bf16)
    # DMAs batched: q two groups per transfer, A/B four groups per transfer.
    wt = [None] * kt
    QB, BB = 2, 4
    qbf_t = [None] * (kt // QB)
    ab_t = [None] * (kt // BB)

    def emit_dequant_group(g):
        if g % BB == 0:
            gb = g // BB
            asb = bcpool.tile([P, BB, N_], bf16, name="asb")
            nc.gpsimd.dma_start(asb[:], a_ap[g:g + BB, :].partition_broadcast(P))
            bsb = bcpool.tile([P, BB, N_], bf16, name="bsb")
            nc.gpsimd.dma_start(bsb[:], b_ap[g:g + BB, :].partition_broadcast(P))
            ab_t[gb] = (asb, bsb)
        if g % QB == 0:
            qsb = qpool.tile([P, QB, N_], u8, name="qsb")
            nc.gpsimd.dma_start(
                qsb[:], qt[g * P:(g + QB) * P, :].rearrange("(ks p) n -> p ks n", p=P))
            qbf = qbpool.tile([P, QB, N_], bf16, name="qbf")
            for ks2 in range(QB):
                nc.scalar.copy(qbf[:, ks2, :], qsb[:, ks2, :])
            qbf_t[g // QB] = qbf
        asb, bsb = ab_t[g // BB]
        qbf = qbf_t[g // QB]
        tmp = tpool.tile([P, N_], bf16, name="tmp")
        nc.vector.tensor_tensor(tmp[:], qbf[:, g % QB, :], asb[:, g % BB, :],
                                mybir.AluOpType.mult)
        w_g = wt_pool.tile([P, N_], bf16, name=f"wt{g}")
        nc.vector.tensor_tensor(w_g[:], tmp[:], bsb[:, g % BB, :],
                                mybir.AluOpType.add)
        wt[g] = w_g

    # GEMM: out[m, n] = sum_k xt[k, m] * wt[k, n] (+ bias via K=1 matmul)
    # Dequant of group g is emitted just before chunk 0 consumes it, so the
    # PE ramps as weights stream in instead of waiting for the full dequant.
    for g in range(QB):
        emit_dequant_group(g)
    nc.sync.dma_start(bias_bc[:], bias_ap[0:1, :].partition_broadcast(P))
    for mc in range(mc_n):
        ps = [[psum.tile([P, 512], f32, name=f"ps_{mi}_{nj}")
               for nj in range(nt)] for mi in range(MS)]
        XB = 4
        for gb in range(kt // XB):
            if mc == 0:
                for ks in range(XB):
                    g2 = gb * XB + ks + QB
                    if g2 < kt:
                        emit_dequant_group(g2)
            xb = xbpool.tile([P, XB, 512], bf16, name="xb")
            nc.sync.dma_start(
                xb[:], xt[gb * XB * P:(gb + 1) * XB * P,
                          mc * 512:(mc + 1) * 512]
                .rearrange("(ks p) m -> p ks m", p=P))
            for ks in range(XB):
                g = gb * XB + ks
                last = g == kt - 1
                for mi in range(MS):
                    for nj in range(nt):
                        nc.tensor.matmul(ps[mi][nj][:],
                                         xb[:, ks, mi * P:(mi + 1) * P],
                                         wt[g][:, nj * 512:(nj + 1) * 512],
                                         start=(g == 0), stop=last)
        if mc == mc_n - 1:
            for mi in range(MS):
                for nj in range(nt):
                    o2 = opool.tile([P, 512], f32, name="o2")
                    nc.vector.tensor_tensor(o2[:], ps[mi][nj][:],
                                            bias_bc[:, nj * 512:(nj + 1) * 512],
                                            mybir.AluOpType.add)
                    nc.sync.dma_start(
                        out_ap[mc * 512 + mi * P: mc * 512 + (mi + 1) * P,
                               nj * 512:(nj + 1) * 512], o2[:])
        else:
            for mi in range(MS):
                o = opool.tile([P, N_], f32, name="o")
                for nj in range(nt):
                    nc.vector.tensor_tensor(o[:, nj * 512:(nj + 1) * 512],
                                            ps[mi][nj][:],
                                            bias_bc[:, nj * 512:(nj + 1) * 512],
                                            mybir.AluOpType.add)
                nc.sync.dma_start(
                    out_ap[mc * 512 + mi * P: mc * 512 + (mi + 1) * P, :], o[:])


def _build_graph():
    nc = bacc.Bacc("TRN2", target_bir_lowering=False, debug=False,
                   num_devices=NCORES)
    f32, bf16, u8 = mybir.dt.float32, mybir.dt.bfloat16, mybir.dt.uint8
    xt_t = nc.dram_tensor("xt", [K, M], bf16, kind="ExternalInput")
    qt_t = nc.dram_tensor("qt", [K, N], u8, kind="ExternalInput")
    a_t = nc.dram_tensor("scl", [KT, N], bf16, kind="ExternalInput")
    b_t = nc.dram_tensor("sclz", [KT, N], bf16, kind="ExternalInput")
    bias_t = nc.dram_tensor("bias", [1, N], f32, kind="ExternalInput")
    out_t = nc.dram_tensor("out", [M, N], f32, kind="ExternalOutput")

    with tile.TileContext(nc) as tc:
        with ExitStack() as ctx:
            _emit(nc, tc, ctx, xt_t.ap(), qt_t.ap(), a_t.ap(), b_t.ap(),
                  bias_t.ap(), out_t.ap(), K, M, N)
    nc.compile()
    return nc


def _get_graph():
    if "nc" not in _CACHE:
        _CACHE["nc"] = _build_graph()
    return _CACHE["nc"]


# ---------------------------------------------------------------------------
# Entry point
# ---------------------------------------------------------------------------

def kernel(x, qweight, scale, zero, bias, _trace=False, _trace_cores=None):
    global LAST_RESULTS
    x = np.ascontiguousarray(np.asarray(x, dtype=np.float32))
    qweight = np.ascontiguousarray(np.asarray(qweight, dtype=np.int32))
    scale = np.asarray(scale, dtype=np.float32)
    zero = np.asarray(zero, dtype=np.float32)
    bias = np.asarray(bias, dtype=np.float32)

    # Host layout prep
    xf = x.reshape(BATCH * SEQ, IN_F)
    q = _unpack_qweight(qweight)                  # [OUT, IN] u8
    qt_full = np.ascontiguousarray(q.T)           # [IN, OUT] u8
    a_full = np.ascontiguousarray(scale.T).astype(ml_dtypes.bfloat16)  # [32, OUT]
    b_full = np.ascontiguousarray(
        (-(2.0 ** (W_BITS - MIN_PREC)) * scale * zero).T
    ).astype(ml_dtypes.bfloat16)                  # [32, OUT]


    xt_halves = [np.ascontiguousarray(xf[mg * M:(mg + 1) * M].T)
                 .astype(ml_dtypes.bfloat16)
                 for mg in range(MG)]             # each [K, M] bf16

    in_maps = []
    for c in range(NCORES):
        mg, ng = divmod(c, NG)
        nsl = slice(ng * N, (ng + 1) * N)
        in_maps.append({
            "xt": xt_halves[mg],
            "qt": np.ascontiguousarray(qt_full[:, nsl]),
            "scl": np.ascontiguousarray(a_full[:, nsl]),
            "sclz": np.ascontiguousarray(b_full[:, nsl]),
            "bias": np.ascontiguousarray(bias[nsl].reshape(1, N)),
        })

    nc = _get_graph()
    res = run_bass_kernel_spmd(nc, in_maps, core_ids=list(range(NCORES)),
                               trace=_trace, trace_cores=_trace_cores)
    LAST_RESULTS = res

    out_full = np.empty((BATCH * SEQ, OUT_F), dtype=np.float32)
    for c in range(NCORES):
        mg, ng = divmod(c, NG)
        out_full[mg * M:(mg + 1) * M, ng * N:(ng + 1) * N] = res.results[c]["out"]
    return out_full.reshape(BATCH, SEQ, OUT_F)
